# revision 1
# baseline (speedup 1.0000x reference)
"""BEVFormer spatial cross attention (MSDeformAttn3D) kernel for 8 TRN2 NeuronCores.

Sharding: the 10000 BEV queries are split across 8 cores (1250 each, padded to
1280). Each core runs deformable attention for its queries against ALL 6
cameras, so the masked per-camera accumulation is core-local and no collective
is needed. Small projection weights and the value tensor are replicated.

Per-core pipeline:
  1. PE: value projection -> per-(cam,head) feature table [6,4,1401,32] bf16 in DRAM
     (1401 = 28*50 cells + 1 pad cell so x0+1 row-pair reads stay in bounds).
  2. PE: q = query+query_pos projected to offsets (64) + attn logits (32).
  3. DVE/ACT: softmax over points, sampling locations, bilinear weights,
     row-pair cell indices (biased by B=32 to keep floor/mod positive).
  4. GPSIMD indirect DMA: for each (cam,head) gather two 2-cell row-pairs
     (64 bf16 elems) per sample from the DRAM table.
  5. DVE: multiply by folded weights (bilinear*attn*mask/count) and reduce.
  6. PE: transpose slots, output projection, +bias +residual.
"""

import sys

for _p in ("/opt/trn_rl_repo",):
    if _p not in sys.path:
        sys.path.insert(0, _p)

import os

import numpy as np

import concourse.bass as bass
import concourse.mybir as mybir
import concourse.tile as tile
from concourse import bacc
from concourse.bass_utils import run_bass_kernel_spmd
from concourse.masks import make_identity

F32 = mybir.dt.float32
BF16 = mybir.dt.bfloat16
I32 = mybir.dt.int32
I16 = mybir.dt.int16

S, N, C, D = 6, 10000, 128, 4
H, P, Pd = 4, 8, 2
HF, WF = 28, 50
M = HF * WF
MP = M + 1  # padded cell count
DH = C // H  # 32
NCORES = 8
NQ = N // NCORES  # 1250
NP = 1280  # padded per-core queries
NS = NP // 128  # 10 supertiles
B = 32.0  # positivity bias for floor/mod trick
IB = int(B)

AluOp = mybir.AluOpType
ActFn = mybir.ActivationFunctionType


def build_nc():
    ablate = set(os.environ.get("KERNEL_ABLATE", "").split(","))
    nc = bacc.Bacc("TRN2", target_bir_lowering=False, debug=False)

    # ---- I/O ----
    qT = nc.declare_dram_parameter("qT", [C, NP], F32, isOutput=False)
    qposT = nc.declare_dram_parameter("qposT", [C, NP], F32, isOutput=False)
    refT = nc.declare_dram_parameter("refT", [S, 2, 128, NS * D], F32, isOutput=False)
    maskT = nc.declare_dram_parameter("maskT", [S, 128, NS * D], F32, isOutput=False)
    valueT = nc.declare_dram_parameter("valueT", [C, S * M], F32, isOutput=False)
    w_oa = nc.declare_dram_parameter("w_oa", [C, 96], F32, isOutput=False)
    b_oa_rep = nc.declare_dram_parameter("b_oa_rep", [C, 96], F32, isOutput=False)
    w_val = nc.declare_dram_parameter("w_val", [C, C], F32, isOutput=False)
    b_val_col = nc.declare_dram_parameter("b_val_col", [C, 1], F32, isOutput=False)
    w_out = nc.declare_dram_parameter("w_out", [C, C], F32, isOutput=False)
    b_out_col = nc.declare_dram_parameter("b_out_col", [C, 1], F32, isOutput=False)
    outT = nc.declare_dram_parameter("outT", [C, NP], F32, isOutput=True)

    with tile.TileContext(nc) as tc:
        with (
            tc.tile_pool(name="persist", bufs=1) as pp,
            tc.tile_pool(name="work", bufs=2) as wp,
            tc.tile_pool(name="gbuf", bufs=2) as gp,
            tc.tile_pool(name="psum", bufs=2, space="PSUM") as psp,
            tc.tile_pool(name="psum1", bufs=1, space="PSUM") as psp1,
            tc.tile_pool(name="dram", bufs=1, space="DRAM") as dp,
        ):
            # quad table: row m of unit (s,h) = [v[m], v[m+1], v[m+50], v[m+51]]
            vtab = dp.tile([S * H * M, 4 * DH], BF16)

            # ---------- load persistent inputs ----------
            qT_sb = pp.tile([C, NP], F32)
            qposT_sb = pp.tile([C, NP], F32)
            nc.sync.dma_start(qT_sb[:], qT[:])
            nc.sync.dma_start(qposT_sb[:], qposT[:])
            w_oa_sb = pp.tile([C, 96], F32)
            b_oa_sb = pp.tile([C, 96], F32)
            w_val_sb = pp.tile([C, C], F32)
            b_val_sb = pp.tile([C, 1], F32)
            w_out_sb = pp.tile([C, C], F32)
            b_out_sb = pp.tile([C, 1], F32)
            nc.sync.dma_start(w_oa_sb[:], w_oa[:])
            nc.sync.dma_start(b_oa_sb[:], b_oa_rep[:])
            nc.sync.dma_start(w_val_sb[:], w_val[:])
            nc.sync.dma_start(b_val_sb[:], b_val_col[:])
            nc.sync.dma_start(w_out_sb[:], w_out[:])
            nc.sync.dma_start(b_out_sb[:], b_out_col[:])

            qsumT = pp.tile([C, NP], F32)
            nc.vector.tensor_add(qsumT[:], qT_sb[:], qposT_sb[:])

            # ---------- value projection -> DRAM quad table ----------
            # vtab row (s,h,m) slots: [v[m], v[m+1], v[m+50], v[m+51]]
            QW = 4 * DH  # 128 elems per quad row
            vt_v = vtab[:].rearrange("(u m) c -> u m c", m=M)
            zq = pp.tile([C, 64], BF16)
            nc.vector.memset(zq[:], 0.0)
            for s in range(S):
                vchunk = wp.tile([C, M], F32, tag="vchunk")
                nc.sync.dma_start(vchunk[:], valueT[:, s * M : (s + 1) * M])
                for off, w in ((0, 512), (512, 512), (1024, 376)):
                    pv = psp.tile([C, 512], F32, tag="pv")
                    nc.tensor.matmul(
                        pv[:, :w],
                        lhsT=w_val_sb[:],
                        rhs=vchunk[:, off : off + w],
                        start=True,
                        stop=True,
                    )
                    vb = wp.tile([C, 512], BF16, tag="vb")
                    nc.scalar.activation(
                        vb[:, :w], pv[:, :w], ActFn.Identity, bias=b_val_sb[:]
                    )
                    # cells [off, off+w) of camera s -> 4 shifted slot writes per head
                    for h in range(H):
                        u = s * H + h
                        vbh = vb[h * DH : (h + 1) * DH, :w]
                        for slot, sh in ((0, 0), (1, 1), (2, WF), (3, WF + 1)):
                            # quad[m][slot] = v[m+sh]: rows m = cell-sh for cells in chunk
                            r0 = off - sh
                            c0 = 0
                            ww = w
                            if r0 < 0:
                                c0 = -r0
                                ww = w + r0
                                r0 = 0
                            if r0 + ww > M:
                                ww = M - r0
                            if ww <= 0:
                                continue
                            nc.sync.dma_start(
                                vt_v[u, r0 : r0 + ww, slot * DH : (slot + 1) * DH]
                                .rearrange("m c -> c m"),
                                vbh[:, c0 : c0 + ww],
                            )
            # zero the out-of-range tail slots (weight-0 but must be finite)
            for s in range(S):
                for h in range(H):
                    u = s * H + h
                    # slot1 @ row M-1 (m+1 == M)
                    nc.sync.dma_start(
                        vt_v[u, M - 1 : M, DH : 2 * DH].rearrange("m c -> c m"),
                        zq[0:DH, 0:1],
                    )
                    # slot2 rows M-WF..M-1, slot3 rows M-WF-1..M-1
                    nc.sync.dma_start(
                        vt_v[u, M - WF : M, 2 * DH : 3 * DH].rearrange("m c -> c m"),
                        zq[0:DH, 0:WF],
                    )
                    nc.sync.dma_start(
                        vt_v[u, M - WF - 1 : M, 3 * DH : 4 * DH].rearrange(
                            "m c -> c m"
                        ),
                        zq[0:DH, 0 : WF + 1],
                    )

            # ---------- offset/attn projection ----------
            off_sb = pp.tile([128, NS * 64], F32)
            off_v = off_sb[:].rearrange("p (n c) -> p n c", c=64)
            latn = pp.tile([128, NS * 32], F32)
            latn_v = latn[:].rearrange("p (n c) -> p n c", c=32)
            for j in range(NS):
                poa = psp.tile([128, 96], F32, tag="poa")
                nc.tensor.matmul(
                    poa[:],
                    lhsT=qsumT[:, j * 128 : (j + 1) * 128],
                    rhs=w_oa_sb[:],
                    start=True,
                    stop=True,
                )
                nc.vector.tensor_add(off_v[:, j, :], poa[:, 0:64], b_oa_sb[:, 0:64])
                nc.vector.tensor_add(latn_v[:, j, :], poa[:, 64:96], b_oa_sb[:, 64:96])

            # ---------- softmax over points (per n,h) ----------
            logit_v = latn[:].rearrange("p (a q) -> p a q", q=P)  # a=(n,h)
            mx = pp.tile([128, NS * H], F32)
            nc.vector.tensor_reduce(
                mx[:].rearrange("p (a o) -> p a o", o=1),
                logit_v,
                axis=mybir.AxisListType.X,
                op=AluOp.max,
            )
            e_sb = pp.tile([128, NS * H * P], F32)
            e_q = e_sb[:].rearrange("p (a q) -> p a q", q=P)
            nc.vector.tensor_tensor(
                e_q, logit_v, mx[:].to_broadcast([128, NS * H, P]), op=AluOp.subtract
            )
            nc.scalar.activation(e_sb[:], e_sb[:], ActFn.Exp)
            ssum = pp.tile([128, NS * H], F32)
            nc.vector.tensor_reduce(
                ssum[:].rearrange("p (a o) -> p a o", o=1),
                e_q,
                axis=mybir.AxisListType.X,
                op=AluOp.add,
            )
            rinv = pp.tile([128, NS * H], F32)
            nc.vector.reciprocal(rinv[:], ssum[:])

            # ---------- bev mask / count ----------
            cnt = pp.tile([128, NS], F32)
            maskS = pp.tile([128, S * NS], F32)
            maskS_v = maskS[:].rearrange("p (s n) -> p s n", s=S)
            for s in range(S):
                mt = wp.tile([128, NS * D], F32, tag="mt")
                nc.sync.dma_start(mt[:], maskT[s])
                nc.vector.tensor_reduce(
                    maskS_v[:, s, :],
                    mt[:].rearrange("p (n d) -> p n d", d=D),
                    axis=mybir.AxisListType.X,
                    op=AluOp.max,
                )
                if s == 0:
                    nc.vector.tensor_copy(cnt[:], maskS_v[:, 0, :])
                else:
                    nc.vector.tensor_add(cnt[:], cnt[:], maskS_v[:, s, :])
            cnt1 = pp.tile([128, NS], F32)
            nc.vector.tensor_scalar_max(cnt1[:], cnt[:], 1.0)
            cinv = pp.tile([128, NS], F32)
            nc.vector.reciprocal(cinv[:], cnt1[:])

            # ---------- per camera: sampling prep + gather + combine ----------
            acc = pp.tile([128, NS * H * DH], F32)  # (nsup, h, c)
            nc.vector.memset(acc[:], 0.0)
            acc_v = acc[:].rearrange("p (n h c) -> p h n c", h=H, c=DH)

            NSAMP = NS * H * P  # 320 free elems per partition
            for s in range(S):
                rx = wp.tile([128, NS * D], F32, tag="rx")
                ry = wp.tile([128, NS * D], F32, tag="ry")
                nc.sync.dma_start(rx[:], refT[s, 0])
                nc.sync.dma_start(ry[:], refT[s, 1])
                def t320(tag):
                    return wp.tile([128, NSAMP], F32, tag=tag, name=tag)

                ixb = t320("ixb")
                iyb = t320("iyb")
                # per-nsup 3-dim STT: ix = ref*W + off  (sample cols (h,pd,d))
                HQ = H * Pd
                off_hq = off_v.rearrange(
                    "p n (hq d two) -> p n hq d two", d=D, two=2
                )
                for j in range(NS):
                    rxj = (
                        rx[:, j * D : (j + 1) * D]
                        .to_broadcast([128, D, HQ])
                        .rearrange("p d e -> p e d")
                    )
                    ryj = (
                        ry[:, j * D : (j + 1) * D]
                        .to_broadcast([128, D, HQ])
                        .rearrange("p d e -> p e d")
                    )
                    ixj = ixb[:, j * 32 : (j + 1) * 32].rearrange(
                        "p (e d) -> p e d", d=D
                    )
                    iyj = iyb[:, j * 32 : (j + 1) * 32].rearrange(
                        "p (e d) -> p e d", d=D
                    )
                    nc.vector.scalar_tensor_tensor(
                        ixj, rxj, float(WF), off_hq[:, j, :, :, 0],
                        op0=AluOp.mult, op1=AluOp.add,
                    )
                    nc.vector.scalar_tensor_tensor(
                        iyj, ryj, float(HF), off_hq[:, j, :, :, 1],
                        op0=AluOp.mult, op1=AluOp.add,
                    )
                # floor via round-to-nearest (x+2^23-2^23) then correct by (round>x)
                RC = 8388608.0
                xr = t320("xr")
                nc.vector.tensor_scalar(
                    xr[:], ixb[:], RC, RC, op0=AluOp.add, op1=AluOp.subtract
                )
                yr = t320("yr")
                nc.vector.tensor_scalar(
                    yr[:], iyb[:], RC, RC, op0=AluOp.add, op1=AluOp.subtract
                )
                xg = t320("xg")
                nc.vector.tensor_tensor(xg[:], xr[:], ixb[:], op=AluOp.is_gt)
                yg = t320("yg")
                nc.vector.tensor_tensor(yg[:], yr[:], iyb[:], op=AluOp.is_gt)
                x0b = t320("x0b")
                nc.vector.tensor_sub(x0b[:], xr[:], xg[:])
                y0b = t320("y0b")
                nc.vector.tensor_sub(y0b[:], yr[:], yg[:])
                fx1 = t320("fx1")
                nc.vector.tensor_sub(fx1[:], ixb[:], x0b[:])
                fy1 = t320("fy1")
                nc.vector.tensor_sub(fy1[:], iyb[:], y0b[:])
                fx0 = t320("fx0")
                nc.vector.tensor_scalar(
                    fx0[:], fx1[:], -1.0, 1.0, op0=AluOp.mult, op1=AluOp.add
                )
                fy0 = t320("fy0")
                nc.vector.tensor_scalar(
                    fy0[:], fy1[:], -1.0, 1.0, op0=AluOp.mult, op1=AluOp.add
                )
                xc = t320("xc")
                nc.vector.tensor_scalar(
                    xc[:], x0b[:], B, B + WF - 1.0, op0=AluOp.max, op1=AluOp.min
                )
                inx0 = t320("inx0")
                nc.vector.tensor_tensor(inx0[:], xc[:], x0b[:], op=AluOp.is_equal)
                x1m = t320("x1m")
                nc.vector.tensor_scalar(
                    x1m[:], x0b[:], B - 1.0, B + WF - 2.0, op0=AluOp.max, op1=AluOp.min
                )
                inx1 = t320("inx1")
                nc.vector.tensor_tensor(inx1[:], x1m[:], x0b[:], op=AluOp.is_equal)
                yc0 = t320("yc0")
                nc.vector.tensor_scalar(
                    yc0[:], y0b[:], B, B + HF - 1.0, op0=AluOp.max, op1=AluOp.min
                )
                iny0 = t320("iny0")
                nc.vector.tensor_tensor(iny0[:], yc0[:], y0b[:], op=AluOp.is_equal)
                y1m = t320("y1m")
                nc.vector.tensor_scalar(
                    y1m[:], y0b[:], B - 1.0, B + HF - 2.0, op0=AluOp.max, op1=AluOp.min
                )
                iny1 = t320("iny1")
                nc.vector.tensor_tensor(iny1[:], y1m[:], y0b[:], op=AluOp.is_equal)
                # slot weights with x0=-1 flip correction
                t1 = t320("t1")
                nc.vector.tensor_mul(t1[:], fx1[:], inx1[:])
                ws1 = t320("ws1")
                nc.vector.tensor_mul(ws1[:], t1[:], inx0[:])
                t2 = t320("t2")
                nc.vector.tensor_mul(t2[:], fx0[:], inx0[:])
                u = t320("u")
                nc.vector.tensor_sub(u[:], t1[:], ws1[:])
                ws0 = t320("ws0")
                nc.vector.tensor_add(ws0[:], t2[:], u[:])
                t1y = t320("t1y")
                nc.vector.tensor_mul(t1y[:], fy1[:], iny1[:])
                wyU = t320("wyU")
                nc.vector.tensor_mul(wyU[:], t1y[:], iny0[:])
                t2y = t320("t2y")
                nc.vector.tensor_mul(t2y[:], fy0[:], iny0[:])
                uy = t320("uy")
                nc.vector.tensor_sub(uy[:], t1y[:], wyU[:])
                wyL = t320("wyL")
                nc.vector.tensor_add(wyL[:], t2y[:], uy[:])
                # quad cell index per sample: m = yB*WF + xLeft - 51*B (table-local)
                # layout (h, n, q)-major for the wrapped-index repack
                mqf = wp.tile([128, NSAMP], F32, tag="mqf", name="mqf")
                mqi = wp.tile([128, NSAMP], I16, tag="mqi", name="mqi")
                yc0_v = yc0[:].rearrange("p (n h q) -> p n h q", h=H, q=P)
                xc_v = xc[:].rearrange("p (n h q) -> p n h q", h=H, q=P)
                mqf_v = mqf[:].rearrange("p (h n q) -> p h n q", h=H, q=P)
                mqi_v = mqi[:].rearrange("p (h n q) -> p h n q", h=H, q=P)
                for h in range(H):
                    nc.vector.scalar_tensor_tensor(
                        mqf_v[:, h],
                        yc0_v[:, :, h, :],
                        float(WF),
                        xc_v[:, :, h, :],
                        op0=AluOp.mult,
                        op1=AluOp.add,
                    )
                    nc.vector.tensor_scalar(
                        mqi_v[:, h], mqf_v[:, h], float(-51 * IB), None,
                        op0=AluOp.add,
                    )

                # folded attention weight: e * recip_sum * mask/count
                ar = wp.tile([128, NS * H], F32, tag="ar")
                a_s = wp.tile([128, NS], F32, tag="a_s")
                nc.vector.tensor_mul(a_s[:], maskS_v[:, s, :], cinv[:])
                nc.vector.tensor_mul(
                    ar[:].rearrange("p (n h) -> p n h", h=H),
                    rinv[:].rearrange("p (n h) -> p n h", h=H),
                    a_s[:].to_broadcast([128, NS, H]),
                )
                awm = t320("awm")
                nc.vector.tensor_mul(
                    awm[:].rearrange("p (a q) -> p a q", q=P),
                    e_q,
                    ar[:].to_broadcast([128, NS * H, P]),
                )
                war0 = t320("war0")
                nc.vector.tensor_mul(war0[:], wyL[:], awm[:])
                war1 = t320("war1")
                nc.vector.tensor_mul(war1[:], wyU[:], awm[:])

                # W tile: (h, nsup, q, slot4) bf16 - matches gathered quad order
                W_s = wp.tile([128, H * NS * P * 4], BF16, tag="W_s")
                W_v = W_s[:].rearrange(
                    "p (h n q t) -> p h n q t", h=H, q=P, t=4
                )
                war0_v = war0[:].rearrange("p (n h q) -> p n h q", h=H, q=P)
                war1_v = war1[:].rearrange("p (n h q) -> p n h q", h=H, q=P)
                ws0_v = ws0[:].rearrange("p (n h q) -> p n h q", h=H, q=P)
                ws1_v = ws1[:].rearrange("p (n h q) -> p n h q", h=H, q=P)
                for h in range(H):
                    for (t, wa, wx) in (
                        (0, war0_v, ws0_v),
                        (1, war0_v, ws1_v),
                        (2, war1_v, ws0_v),
                        (3, war1_v, ws1_v),
                    ):
                        nc.vector.tensor_mul(
                            W_v[:, h, :, :, t], wa[:, :, h, :], wx[:, :, h, :]
                        )

                # repack indices into dma_gather wrapped format:
                # wrapped[p%16, h*640 + (n*8+q)*8 + p//16] = m[p, (h,n,q)]
                wrp = wp.tile([128, H * NS * P * 8], I16, tag="wrp", name="wrp")
                for pg in range(8):
                    nc.sync.dma_start(
                        wrp[0:16, :]
                        .rearrange("p (a e) -> p a e", e=8)[:, :, pg]
                        .rearrange("p (hn q) -> p hn q", q=P),
                        mqi[pg * 16 : (pg + 1) * 16, :],
                    )
                # replicate to all 8 Q7 core groups
                for rep in range(1, 8):
                    nc.sync.dma_start(
                        wrp[rep * 16 : (rep + 1) * 16, :], wrp[0:16, :]
                    )

                for h in range(H):
                    # gathered quads: (nsup*q, slot4*c32)
                    g = gp.tile([128, NS * P * 4 * DH], BF16, tag="g")
                    u = s * H + h
                    if "gather" not in ablate:
                        nc.gpsimd.dma_gather(
                            out_ap=g[:].rearrange("p (j c) -> p j c", c=4 * DH),
                            in_ap=vtab[u * M : (u + 1) * M, :],
                            idxs_ap=wrp[:, h * 640 : (h + 1) * 640],
                            num_idxs=NS * P * 128,
                            num_idxs_reg=NS * P * 128,
                            elem_size=4 * DH,
                            single_packet=False,
                        )
                    # weighted multiply (in place): a=(n,q,slot) 320 terms x 32c
                    if "combine" not in ablate:
                        W_b = W_v[:, h].rearrange(
                            "p n q t -> p (n q t)"
                        ).to_broadcast([128, NS * P * 4, DH])
                        g_a = g[:].rearrange("p (a c) -> p a c", c=DH)
                        nc.vector.tensor_tensor(g_a, g_a, W_b, op=AluOp.mult)
                        # reduce over (q,slot) keeping (nsup, c)
                        red = wp.tile([128, NS * DH], F32, tag="red")
                        nc.vector.tensor_reduce(
                            red[:].rearrange("p (n c) -> p n c", c=DH),
                            g[:].rearrange(
                                "p (n qt c) -> p n c qt", qt=P * 4, c=DH
                            ),
                            axis=mybir.AxisListType.X,
                            op=AluOp.add,
                        )
                        nc.vector.tensor_add(
                            acc_v[:, h],
                            acc_v[:, h],
                            red[:].rearrange("p (n c) -> p n c", c=DH),
                        )

            # ---------- transpose slots + output projection ----------
            ident = pp.tile([128, 128], F32)
            make_identity(nc, ident)
            slotsT = pp.tile([C, NP], F32)
            for j in range(NS):
                pt = psp.tile([128, 128], F32, tag="pt")
                nc.tensor.transpose(
                    pt[:],
                    in_=acc[:, j * 128 : (j + 1) * 128],
                    identity=ident[:],
                )
                nc.scalar.copy(slotsT[:, j * 128 : (j + 1) * 128], pt[:])
            fo = pp.tile([C, NP], F32)
            for off, w in ((0, 512), (512, 512), (1024, 256)):
                pf = psp.tile([128, 512], F32, tag="pf")
                nc.tensor.matmul(
                    pf[:, :w],
                    lhsT=w_out_sb[:],
                    rhs=slotsT[:, off : off + w],
                    start=True,
                    stop=True,
                )
                nc.scalar.activation(
                    fo[:, off : off + w], pf[:, :w], ActFn.Identity, bias=b_out_sb[:]
                )
            nc.vector.tensor_add(fo[:], fo[:], qT_sb[:])
            nc.sync.dma_start(outT[:], fo[:])

    nc.compile()
    return nc


_NC_CACHE = None


def _get_nc():
    global _NC_CACHE
    if _NC_CACHE is None:
        _NC_CACHE = build_nc()
    return _NC_CACHE


def make_in_maps(inputs):
    query = np.asarray(inputs["query"], np.float32)  # (1, N, C)
    key = inputs["key"]  # unused (shape source only)
    value = np.asarray(inputs["value"], np.float32)  # (S, M, 1, C)
    query_pos = np.asarray(inputs["query_pos"], np.float32)
    ref = np.asarray(inputs["reference_points_cam"], np.float32)  # (S,1,N,D,2)
    bev_mask = np.asarray(inputs["bev_mask"])  # (S,1,N,D) bool
    w_off = np.asarray(inputs["w_off"], np.float32)
    b_off = np.asarray(inputs["b_off"], np.float32)
    w_attn = np.asarray(inputs["w_attn"], np.float32)
    b_attn = np.asarray(inputs["b_attn"], np.float32)
    w_val = np.asarray(inputs["w_val"], np.float32)
    b_val = np.asarray(inputs["b_val"], np.float32)
    w_out = np.asarray(inputs["w_out"], np.float32)
    b_out = np.asarray(inputs["b_out"], np.float32)

    w_oa = np.concatenate([w_off, w_attn], axis=1)  # (C, 96)
    # fold -0.5 (align_corners) and +B (positivity) into the offset bias
    b_eff = b_off.copy().reshape(-1, 2)
    b_eff += np.float32(B - 0.5)
    b_oa = np.concatenate([b_eff.reshape(-1), b_attn])  # (96,)
    b_oa_rep = np.broadcast_to(b_oa[None, :], (C, 96)).copy()

    valueT = value[:, :, 0, :].reshape(S * M, C).T.copy()  # (C, S*M)

    in_maps = []
    for k in range(NCORES):
        n0, n1 = k * NQ, (k + 1) * NQ
        qT = np.zeros((C, NP), np.float32)
        qT[:, :NQ] = query[0, n0:n1].T
        qposT = np.zeros((C, NP), np.float32)
        qposT[:, :NQ] = query_pos[0, n0:n1].T
        refp = np.zeros((S, 2, NP, D), np.float32)
        refp[:, 0, :NQ, :] = ref[:, 0, n0:n1, :, 0]
        refp[:, 1, :NQ, :] = ref[:, 0, n0:n1, :, 1]
        # (s, xy, n, d) -> (s, xy, p, (nsup d))
        refT = refp.reshape(S, 2, NS, 128, D).transpose(0, 1, 3, 2, 4).reshape(
            S, 2, 128, NS * D
        ).copy()
        maskp = np.zeros((S, NP, D), np.float32)
        maskp[:, :NQ, :] = bev_mask[:, 0, n0:n1, :].astype(np.float32)
        maskT = maskp.reshape(S, NS, 128, D).transpose(0, 2, 1, 3).reshape(
            S, 128, NS * D
        ).copy()
        in_maps.append(
            {
                "qT": qT,
                "qposT": qposT,
                "refT": refT,
                "maskT": maskT,
                "valueT": valueT,
                "w_oa": w_oa,
                "b_oa_rep": b_oa_rep,
                "w_val": w_val,
                "b_val_col": b_val.reshape(C, 1).copy(),
                "w_out": w_out,
                "b_out_col": b_out.reshape(C, 1).copy(),
            }
        )
    return in_maps


def kernel(**inputs):
    nc = _get_nc()
    in_maps = make_in_maps(inputs)
    res = run_bass_kernel_spmd(nc, in_maps, core_ids=list(range(NCORES)))
    out = np.zeros((1, N, C), np.float32)
    for k in range(NCORES):
        out[0, k * NQ : (k + 1) * NQ] = res.results[k]["outT"].T[:NQ]
    return out


if __name__ == "__main__":
    nc = build_nc()
    print("built ok")

